# revision 52
# baseline (speedup 1.0000x reference)
"""Multi-head attention (B=4, N=2048, D=1024, H=16) on 8 Trainium2 NeuronCores.

Sharding: core = (batch b = core//2, head-group g = core%2 of 8 heads).
Each core computes qkv + attention for its 8 heads and a *partial* output
projection over its 512 features; the host sums the two partials per batch
and adds the bias (the tensor-parallel unshard).

All matmuls run in fp32r (TF32-like, full PE speed at moving dim >=256).
Scores are computed transposed (S^T[m,n]: keys on partitions) so softmax
needs no on-chip transpose; a ones-column appended to v yields the softmax
denominators inside the same PE accumulation as attn@v.

Score matmuls run at K=128 (never K=64, which measures ~2x slower per
instruction on TRN2): the pair's two heads stack their 64 k-dims on the
partition axis of kT, and q is stored zero-padded per head so the cross-
head contraction terms vanish.

Emission is software-pipelined for the ACT engine (exp is the per-core
roofline: 33.5M elements at 1 elem/lane/cycle, plus ~0.9us semaphore
latency per cross-engine hop): scores use a 2-deep [128,1024] PSUM ring,
exp consumes 1024 columns per instruction, attn@v trails 3 steps behind,
and one flat step stream runs across ALL head-pairs and n-chunks so the
exp cadence never breaks at a boundary. Epilogues (softmax normalization)
trail 3+4 steps behind their accumulator so the 3.3us DVE reciprocal never
blocks the PE; qT(j+1) and the projection of chunk j-1 interleave as
background thunks in a dedicated 2-bank PSUM pool (proj delayed 16 steps
so at(j-1) is fully written in program order first).
"""
import sys

sys.path.insert(0, '/opt/trn_rl_repo')

import numpy as np

import concourse.bass as bass  # noqa: F401  (registers engines)
import concourse.mybir as mybir
import concourse.tile as tile
from concourse import bacc
from concourse.bass_utils import run_bass_kernel_spmd

dt = mybir.dt

B = 4
N = 2048          # sequence length
D = 1024          # d_model
NH = 16           # total heads
HD = 64           # head dim
NHC = 8           # heads per core
DC = NHC * HD     # 512 features per core
SCALE = HD ** -0.5

P = 128           # partitions
BG_INTERLEAVE = True
KB = D // P       # 8 k-blocks
NCH = N // 512    # 4 n-chunks of 512
MT = N // P       # 16 m-tiles of 128
DB = DC // P      # 4 d'-blocks / c-blocks


def build_program(debug=False):
    nc = bacc.Bacc("TRN2", target_bir_lowering=False, debug=False,
                   enable_asserts=False, num_devices=8)

    xT = nc.dram_tensor("xT", [D, N], dt.float32, kind="ExternalInput")
    wqT = nc.dram_tensor("wqT", [D, DC], dt.float32, kind="ExternalInput")
    wkT = nc.dram_tensor("wkT", [D, DC], dt.float32, kind="ExternalInput")
    wvT = nc.dram_tensor("wvT", [D, DC], dt.float32, kind="ExternalInput")
    wpT = nc.dram_tensor("wpT", [DC, D], dt.float32, kind="ExternalInput")
    out = nc.dram_tensor("out", [N, D], dt.float32, kind="ExternalOutput")

    f32r = dt.float32r
    f32 = dt.float32
    Exp = mybir.ActivationFunctionType.Exp
    MULT = mybir.AluOpType.mult
    DIV = mybir.AluOpType.divide

    with tile.TileContext(nc) as tc:
        with tc.tile_pool(name="persist", bufs=1) as persist, \
             tc.tile_pool(name="wq", bufs=1) as wq_pool, \
             tc.tile_pool(name="qTc", bufs=2) as qT_pool, \
             tc.tile_pool(name="ps_S", bufs=2, space="PSUM") as ps_S, \
             tc.tile_pool(name="ps_bg", bufs=2, space="PSUM") as ps_bg, \
             tc.tile_pool(name="ps_o", bufs=1, space="PSUM") as ps_o:

            # ---- persistent SBUF tensors ----
            kT_sb = persist.tile([P, DB, N], f32r, tag="kT")
            # v with a ones column per head: [m-part, m-tile, head, 65]
            v_sb = persist.tile([P, MT, NHC, HD + 1], f32r, tag="v")
            ones_sb = persist.tile([P, HD], f32r, tag="ones")

            wq_sb = wq_pool.tile([P, KB, DC], f32r, tag="wq")
            nc.vector.memset(v_sb[:].bitcast(f32), 1.0)
            nc.vector.memset(ones_sb[:].bitcast(f32), 1.0)

            xw_pool_box = [None]

            def load_xw(j, label, fine=False):
                xw = xw_pool_box[0].tile([P, KB, 512], f32r, tag="xw",
                                         name=f"xw_{label}")
                ap = (xT.ap()[:, j * 512:(j + 1) * 512]
                      .rearrange("(kb p) n -> p kb n", p=P).bitcast(f32r))
                if fine:
                    # per-k-block DMAs so matmuls start on partial data
                    for kb in range(KB):
                        nc.sync.dma_start(xw[:, kb, :], ap[:, kb, :])
                else:
                    nc.sync.dma_start(xw[:], ap)
                return xw

            def emit_proj_tiles(xw, w_sb, dst_fn, lbl, copy_fn=None):
                """q/k projection for one 512-window: 4 d'-blocks."""
                for db in range(DB):
                    pq = ps_bg.tile([P, 512], f32, tag="bg",
                                    name=f"pq_{lbl}_{db}")
                    for kb in range(KB):
                        nc.tensor.matmul(
                            pq[:],
                            lhsT=w_sb[:, kb, db * P:(db + 1) * P],
                            rhs=xw[:, kb, :],
                            start=(kb == 0), stop=(kb == KB - 1))
                    if copy_fn is not None:
                        copy_fn(db, pq)
                    else:
                        nc.vector.tensor_copy(out=dst_fn(db), in_=pq[:])

            def emit_v_window(xw, w, wv_sb):
                """v for the 4 m-tiles of window w."""
                for mc in range(4):
                    m = w * 4 + mc
                    pv = ps_bg.tile([P, 512], f32, tag="bg", name=f"pv{m}")
                    for kb in range(KB):
                        nc.tensor.matmul(
                            pv[:],
                            lhsT=xw[:, kb, mc * P:(mc + 1) * P],
                            rhs=wv_sb[:, kb, :],
                            start=(kb == 0), stop=(kb == KB - 1))
                    nc.vector.tensor_copy(
                        out=v_sb[:, m, :, 0:HD],
                        in_=pv[:].rearrange("p (h d) -> p h d", h=NHC))

            qT_tiles = [None] * NCH

            def qt_copy_fn(qt):
                # scores run as K=128 matmuls: per head its q rows live in
                # their own 64 partitions, the other head's 64 rows are zero
                def cp(db, pq):
                    nc.vector.tensor_copy(out=qt[0:HD, 2 * db, :],
                                          in_=pq[0:HD, :])
                    nc.vector.tensor_copy(out=qt[HD:P, 2 * db + 1, :],
                                          in_=pq[HD:P, :])
                return cp

            def emit_qT_chunk(j):
                qt = qT_pool.tile([P, NHC, 512], f32r, tag="qTc", name=f"qT{j}")
                nc.vector.memset(qt[:].bitcast(f32), 0.0)
                xwq = load_xw(j, f"q{j}")
                emit_proj_tiles(xwq, wq_sb, None, f"q{j}",
                                copy_fn=qt_copy_fn(qt))
                qT_tiles[j] = qt

            # ---- prelude: kT + v for all windows, then qT chunk 0 ----
            xwp_scope = tc.tile_pool(name="xwp", bufs=2)
            xw_pool_box[0] = xwp_scope.__enter__()
            wkv_scope = tc.tile_pool(name="wkv", bufs=1)
            wkv_pool = wkv_scope.__enter__()
            wk_sb = wkv_pool.tile([P, KB, DC], f32r, tag="wk")
            wv_sb = wkv_pool.tile([P, KB, DC], f32r, tag="wv")
            wk_ap = wkT.ap().rearrange("(kb p) d -> p kb d", p=P).bitcast(f32r)
            wv_ap = wvT.ap().rearrange("(kb p) d -> p kb d", p=P).bitcast(f32r)
            xw0 = xw_pool_box[0].tile([P, KB, 512], f32r, tag="xw",
                                      name="xw_kv0")
            xw0_ap = (xT.ap()[:, 0:512]
                      .rearrange("(kb p) n -> p kb n", p=P).bitcast(f32r))
            # interleave so the first kT matmul's inputs land first; wv is
            # only needed once kT window 0 is done
            for kb in range(KB):
                nc.sync.dma_start(wk_sb[:, kb, :], wk_ap[:, kb, :])
                nc.sync.dma_start(xw0[:, kb, :], xw0_ap[:, kb, :])
            for kb in range(KB):
                nc.sync.dma_start(wv_sb[:, kb, :], wv_ap[:, kb, :])

            for w in range(NCH):
                xw = xw0 if w == 0 else load_xw(w, f"kv{w}")
                emit_proj_tiles(
                    xw, wk_sb,
                    lambda db, w=w: kT_sb[:, db, w * 512:(w + 1) * 512],
                    f"k{w}")
                emit_v_window(xw, w, wv_sb)
                if w == 0:
                    nc.sync.dma_start(
                        wq_sb[:],
                        wqT.ap().rearrange("(kb p) d -> p kb d",
                                           p=P).bitcast(f32r))
            emit_qT_chunk(0)
            wkv_scope.__exit__(None, None, None)
            xwp_scope.__exit__(None, None, None)

            xw2_scope = tc.tile_pool(name="xw2", bufs=1)
            xw_pool_box[0] = xw2_scope.__enter__()
            expS_scope = tc.tile_pool(name="expS", bufs=7)
            expS_pool = expS_scope.__enter__()
            at_scope = tc.tile_pool(name="at", bufs=2)
            at_pool = at_scope.__enter__()
            small_scope = tc.tile_pool(name="small", bufs=2)
            small_pool = small_scope.__enter__()
            out_scope = tc.tile_pool(name="outsb", bufs=2)
            out_pool = out_scope.__enter__()

            if debug:
                dbg_qT = nc.dram_tensor("dbg_qT", [P, DB, 512], f32, kind="ExternalOutput")
                dbg_kT = nc.dram_tensor("dbg_kT", [P, DB, N], f32, kind="ExternalOutput")
                dbg_v = nc.dram_tensor("dbg_v", [P, MT, NHC, HD + 1], f32, kind="ExternalOutput")
                dbg_at = nc.dram_tensor("dbg_at", [P, DB, 512], f32, kind="ExternalOutput")
                nc.sync.dma_start(dbg_qT.ap(), qT_tiles[0][:].bitcast(f32))

            # ---- attention + projection, per n-chunk ----
            # All score/qT/proj/bcp PSUM traffic shares one 3-deep ring of
            # [128,1024] tiles (6 banks); attn@v accumulators get 2 banks.
            at_tiles = [None] * NCH

            def emit_qT_thunks(j):
                """qT(j) emission as small PE thunks (ring-pool psum)."""
                qt = qT_pool.tile([P, NHC, 512], f32r, tag="qTc", name=f"qT{j}")
                qT_tiles[j] = qt
                xwq = load_xw(j, f"q{j}")
                cpf = qt_copy_fn(qt)
                thunks = [lambda qt=qt: nc.vector.memset(qt[:].bitcast(f32),
                                                         0.0)]
                box = [None]
                for db in range(DB):
                    def mm_t(db, kb0):
                        if kb0 == 0:
                            box[0] = ps_bg.tile([P, 512], f32, tag="bg",
                                                name=f"pqt{db}")
                        for kb in (kb0, kb0 + 1):
                            nc.tensor.matmul(
                                box[0][:],
                                lhsT=wq_sb[:, kb, db * P:(db + 1) * P],
                                rhs=xwq[:, kb, :],
                                start=(kb == 0), stop=(kb == KB - 1))
                    for kb0 in range(0, KB, 2):
                        thunks.append(lambda db=db, kb0=kb0: mm_t(db, kb0))
                    def cp_t(db=db):
                        cpf(db, box[0])
                    thunks.append(cp_t)
                return thunks

            def emit_proj_thunks(j, dual_pool=False):
                """Projection of chunk j as small PE thunks (ring psum).
                dual_pool alternates ps_bg/ps_S tiles (final burst: the S
                ring is idle, 4-way rotation halves the serialization)."""
                at_j = at_tiles[j]
                thunks = []
                box = [None]
                for ns in range(4):
                    for ec in range(2):
                        def mm_t(ns, ec, kb0):
                            if kb0 == 0:
                                if dual_pool and (ns * 2 + ec) % 2 == 1:
                                    box[0] = ps_S.tile(
                                        [P, 1024], f32, tag="S",
                                        name=f"ppt{ns}_{ec}")[:, 0:512]
                                else:
                                    box[0] = ps_bg.tile(
                                        [P, 512], f32, tag="bg",
                                        name=f"ppt{ns}_{ec}")
                            for cb in (kb0, kb0 + 1):
                                nc.tensor.matmul(
                                    box[0][:],
                                    lhsT=at_j[:, cb, ns * P:(ns + 1) * P],
                                    rhs=wp_box[0][:, cb, ec * 512:(ec + 1) * 512],
                                    start=(cb == 0), stop=(cb == DB - 1))
                        for kb0 in range(0, DB, 2):
                            thunks.append(
                                lambda ns=ns, ec=ec, kb0=kb0: mm_t(ns, ec, kb0))
                        def cp_t(ns=ns, ec=ec):
                            osb = out_pool.tile([P, 512], f32, tag="osb",
                                                name=f"osb{ns}_{ec}")
                            nc.vector.tensor_copy(out=osb[:], in_=box[0][:])
                            nc.sync.dma_start(
                                out.ap()[j * 512 + ns * P:j * 512 + (ns + 1) * P,
                                         ec * 512:(ec + 1) * 512],
                                osb[:])
                        thunks.append(cp_t)
                return thunks

            wp_box = [None]

            # ---- one flat step stream across ALL chunks ----
            # S lookahead, attn@v lag and epilogues run continuously across
            # pair AND chunk boundaries; qT(j+1)/proj(j-1) interleave as
            # background thunks (proj delayed past the epilogue flush so
            # at(j-1) is fully written in program order first).
            gsteps = [(j, p, h, i) for j in range(NCH) for p in range(DB)
                      for h in range(2) for i in range(MT // 2)]
            CHUNK = len(gsteps) // NCH
            po_tiles = {}
            eS_q = {}
            pending_p1 = []
            pending_ep = []
            bg_now = []
            bg_later = []
            AV_LAG = 3

            def emit_bg(idx):
                while bg_later and bg_later[0][0] <= idx:
                    bg_now.extend(bg_later.pop(0)[1])
                if bg_now:
                    bg_now.pop(0)()

            def emit_S(j, p, h, i):
                S = ps_S.tile([P, 1024], f32, tag="S", name=f"S{p}_{h}_{i}")
                for half in range(2):
                    m = 2 * i + half
                    nc.tensor.matmul(
                        S[:, half * 512:(half + 1) * 512],
                        lhsT=kT_sb[:, p, m * P:(m + 1) * P],
                        rhs=qT_tiles[j][:, 2 * p + h, :],
                        start=True, stop=True)
                return S

            def emit_epilogue_p1(po_t, h):
                oT = small_pool.tile([HD + 1, 512], f32, tag=f"oT{h}",
                                     name=f"oT{h}")
                nc.vector.tensor_copy(out=oT[:], in_=po_t[0:HD + 1, :])
                rcp = small_pool.tile([HD + 1, 512], f32r, tag="rcp",
                                      name="rcp")
                with nc.allow_low_precision(reason="softmax recip to f32r"):
                    nc.vector.reciprocal(rcp[HD:HD + 1, :], oT[HD:HD + 1, :])
                return oT, rcp

            def emit_epilogue_p2(jj, p, h, oT, rcp):
                at = at_tiles[jj]
                bcp = ps_bg.tile([P, 512], f32, tag="bg", name=f"bcp{h}")
                nc.tensor.matmul(bcp[0:HD, :],
                                 lhsT=ones_sb[HD:HD + 1, :],
                                 rhs=rcp[HD:HD + 1, :],
                                 start=True, stop=True)
                if h == 0:
                    nc.vector.tensor_tensor(
                        out=at[0:HD, p, :], in0=oT[0:HD, :],
                        in1=bcp[0:HD, :], op=MULT)
                else:
                    nc.vector.tensor_tensor(
                        out=oT[0:HD, :], in0=oT[0:HD, :],
                        in1=bcp[0:HD, :], op=MULT)
                    nc.sync.dma_start(at[HD:P, p, :],
                                      oT[0:HD, :].bitcast(f32r))

            def emit_av(idx2):
                jj, pp, ph, pi = gsteps[idx2]
                eSp = eS_q.pop((jj, pp, ph, pi))
                key = (jj, pp)
                if key not in po_tiles:
                    po_tiles[key] = [
                        ps_o.tile([P, 512], f32, tag="o",
                                  name=f"po{jj}_{pp}_{h2}")
                        for h2 in range(2)]
                po_t = po_tiles[key][ph]
                for half in range(2):
                    m = 2 * pi + half
                    nc.tensor.matmul(
                        po_t[0:HD + 1, :],
                        lhsT=v_sb[:, m, 2 * pp + ph, :],
                        rhs=eSp[:, half * 512:(half + 1) * 512],
                        start=(m == 0), stop=(m == MT - 1))
                if pi == MT // 2 - 1:
                    pending_p1.append([3, po_t, jj, pp, ph])

            def drain_queues():
                for ep in pending_p1:
                    ep[0] -= 1
                while pending_p1 and pending_p1[0][0] <= 0:
                    _, po_t, jj, pp, ph = pending_p1.pop(0)
                    oT, rcp = emit_epilogue_p1(po_t, ph)
                    pending_ep.append([4, jj, pp, ph, oT, rcp])
                for ep in pending_ep:
                    ep[0] -= 1
                while pending_ep and pending_ep[0][0] <= 0:
                    _, jj, pp, ph, oT, rcp = pending_ep.pop(0)
                    emit_epilogue_p2(jj, pp, ph, oT, rcp)

            def start_chunk(c, idx):
                if c == 0:
                    wp_scope = tc.tile_pool(name="wp", bufs=1)
                    wp_pool = wp_scope.__enter__()
                    wp_box.append(wp_scope)  # keep scope alive
                    wp_sb = wp_pool.tile([P, DB, D], f32r, tag="wp")
                    nc.sync.dma_start(
                        wp_sb[:],
                        wpT.ap().rearrange("(cb p) e -> p cb e",
                                           p=P).bitcast(f32r))
                    wp_box[0] = wp_sb
                at_tiles[c] = at_pool.tile([P, DB, 512], f32r, tag="at",
                                           name=f"at{c}")
                if c + 1 < NCH:
                    bg_now.extend(emit_qT_thunks(c + 1))
                if c >= 1:
                    bg_later.append((idx + 16, emit_proj_thunks(c - 1)))

            start_chunk(0, 0)
            S_next = emit_S(*gsteps[0])
            for idx, (j, p, h, i) in enumerate(gsteps):
                S_cur = S_next
                eS = expS_pool.tile([P, 1024], f32r, tag="e",
                                    name=f"eS{p}_{h}_{i}")
                nc.scalar.activation(eS[:], S_cur[:], Exp, scale=SCALE)
                eS_q[(j, p, h, i)] = eS
                if idx + 1 < len(gsteps):
                    if (idx + 1) % CHUNK == 0:
                        # entering a new chunk: its qT thunks must be fully
                        # emitted before the lookahead score reads them
                        while bg_now:
                            bg_now.pop(0)()
                        start_chunk((idx + 1) // CHUNK, idx + 1)
                    S_next = emit_S(*gsteps[idx + 1])
                if idx >= AV_LAG:
                    emit_av(idx - AV_LAG)
                drain_queues()
                emit_bg(idx)
                if len(bg_now) > len(gsteps) - idx:
                    emit_bg(idx)

            for idx2 in range(len(gsteps) - AV_LAG, len(gsteps)):
                emit_av(idx2)
            for _, po_t, jj, pp, ph in pending_p1:
                oT, rcp = emit_epilogue_p1(po_t, ph)
                pending_ep.append([0, jj, pp, ph, oT, rcp])
            for _, jj, pp, ph, oT, rcp in pending_ep:
                emit_epilogue_p2(jj, pp, ph, oT, rcp)
            while bg_now or bg_later:
                if not bg_now and bg_later:
                    bg_now.extend(bg_later.pop(0)[1])
                if bg_now:
                    bg_now.pop(0)()

            # final chunk's projection
            for t in emit_proj_thunks(NCH - 1, dual_pool=True):
                t()

            if len(wp_box) > 1:
                wp_box[1].__exit__(None, None, None)
            out_scope.__exit__(None, None, None)
            small_scope.__exit__(None, None, None)
            at_scope.__exit__(None, None, None)
            expS_scope.__exit__(None, None, None)
            xw2_scope.__exit__(None, None, None)

    nc.compile()
    return nc


_CACHE: dict = {}


def _get_program():
    if "nc" not in _CACHE:
        _CACHE["nc"] = build_program()
    return _CACHE["nc"]


def make_in_maps(x, w_qkv, w_proj):
    """Host-side sharding: per-core input dict."""
    x = np.ascontiguousarray(np.asarray(x, dtype=np.float32))
    w_qkv = np.asarray(w_qkv, dtype=np.float32)
    w_proj = np.asarray(w_proj, dtype=np.float32)
    in_maps = []
    for core in range(8):
        b, g = divmod(core, 2)
        gsl = slice(g * DC, (g + 1) * DC)
        in_maps.append({
            "xT": np.ascontiguousarray(x[b].T),                       # [D, N]
            "wqT": np.ascontiguousarray(w_qkv[0 * D:1 * D][gsl].T),   # [D, DC]
            "wkT": np.ascontiguousarray(w_qkv[1 * D:2 * D][gsl].T),
            "wvT": np.ascontiguousarray(w_qkv[2 * D:3 * D][gsl].T),
            "wpT": np.ascontiguousarray(w_proj[:, gsl].T),            # [DC, D]
        })
    return in_maps


def run(x, w_qkv, w_proj, b_proj, **spmd_kwargs):
    nc = _get_program()
    in_maps = make_in_maps(x, w_qkv, w_proj)
    res = run_bass_kernel_spmd(nc, in_maps, list(range(8)), **spmd_kwargs)
    b_proj = np.asarray(b_proj, dtype=np.float32)
    outp = np.empty((B, N, D), dtype=np.float32)
    for b in range(B):
        outp[b] = (res.results[2 * b]["out"] + res.results[2 * b + 1]["out"]
                   + b_proj[None, :])
    return outp, res


def kernel(x, w_qkv, w_proj, b_proj):
    outp, _ = run(x, w_qkv, w_proj, b_proj)
    return outp


# revision 56
# speedup vs baseline: 1.0201x; 1.0201x over previous
"""Multi-head attention (B=4, N=2048, D=1024, H=16) on 8 Trainium2 NeuronCores.

Sharding: core = (batch b = core//2, head-group g = core%2 of 8 heads).
Each core computes qkv + attention for its 8 heads and a *partial* output
projection over its 512 features; the host sums the two partials per batch
and adds the bias (the tensor-parallel unshard).

All matmuls run in fp32r (TF32-like, full PE speed at moving dim >=256).
Scores are computed transposed (S^T[m,n]: keys on partitions) so softmax
needs no on-chip transpose; a ones-column appended to v yields the softmax
denominators inside the same PE accumulation as attn@v.

Score matmuls run at K=128 (never K=64, which measures ~2x slower per
instruction on TRN2): the pair's two heads stack their 64 k-dims on the
partition axis of kT, and q is stored zero-padded per head so the cross-
head contraction terms vanish.

Emission is software-pipelined for the ACT engine (exp is the per-core
roofline: 33.5M elements at 1 elem/lane/cycle, plus ~0.9us semaphore
latency per cross-engine hop): scores use a 2-deep [128,1024] PSUM ring,
exp consumes 1024 columns per instruction, attn@v trails 3 steps behind,
and one flat step stream runs across ALL head-pairs and n-chunks so the
exp cadence never breaks at a boundary. Epilogues (softmax normalization)
trail 3+4 steps behind their accumulator so the 3.3us DVE reciprocal never
blocks the PE; qT(j+1) and the projection of chunk j-1 interleave as
background thunks in a dedicated 2-bank PSUM pool (proj delayed 16 steps
so at(j-1) is fully written in program order first).
"""
import sys

sys.path.insert(0, '/opt/trn_rl_repo')

import numpy as np

import concourse.bass as bass  # noqa: F401  (registers engines)
import concourse.mybir as mybir
import concourse.tile as tile
from concourse import bacc
from concourse.bass_utils import run_bass_kernel_spmd

dt = mybir.dt

B = 4
N = 2048          # sequence length
D = 1024          # d_model
NH = 16           # total heads
HD = 64           # head dim
NHC = 8           # heads per core
DC = NHC * HD     # 512 features per core
SCALE = HD ** -0.5

P = 128           # partitions
BG_INTERLEAVE = True
KB = D // P       # 8 k-blocks
NCH = N // 512    # 4 n-chunks of 512
MT = N // P       # 16 m-tiles of 128
DB = DC // P      # 4 d'-blocks / c-blocks


def build_program(debug=False):
    nc = bacc.Bacc("TRN2", target_bir_lowering=False, debug=False,
                   enable_asserts=False, num_devices=8)

    xT = nc.dram_tensor("xT", [D, N], dt.float32, kind="ExternalInput")
    wqT = nc.dram_tensor("wqT", [D, DC], dt.float32, kind="ExternalInput")
    wkT = nc.dram_tensor("wkT", [D, DC], dt.float32, kind="ExternalInput")
    wvT = nc.dram_tensor("wvT", [D, DC], dt.float32, kind="ExternalInput")
    wpT = nc.dram_tensor("wpT", [DC, D], dt.float32, kind="ExternalInput")
    out = nc.dram_tensor("out", [N, D], dt.float32, kind="ExternalOutput")

    f32r = dt.float32r
    f32 = dt.float32
    Exp = mybir.ActivationFunctionType.Exp
    MULT = mybir.AluOpType.mult
    DIV = mybir.AluOpType.divide

    with tile.TileContext(nc) as tc:
        with tc.tile_pool(name="persist", bufs=1) as persist, \
             tc.tile_pool(name="wq", bufs=1) as wq_pool, \
             tc.tile_pool(name="qTc", bufs=2) as qT_pool, \
             tc.tile_pool(name="xw", bufs=2) as xw_pool, \
             tc.tile_pool(name="ps_S", bufs=2, space="PSUM") as ps_S, \
             tc.tile_pool(name="ps_bg", bufs=2, space="PSUM") as ps_bg, \
             tc.tile_pool(name="ps_o", bufs=1, space="PSUM") as ps_o:

            # ---- persistent SBUF tensors ----
            kT_sb = persist.tile([P, DB, N], f32r, tag="kT")
            # v with a ones column per head: [m-part, m-tile, head, 65]
            v_sb = persist.tile([P, MT, NHC, HD + 1], f32r, tag="v")
            ones_sb = persist.tile([P, HD], f32r, tag="ones")

            wq_sb = wq_pool.tile([P, KB, DC], f32r, tag="wq")
            nc.vector.memset(v_sb[:].bitcast(f32), 1.0)
            nc.vector.memset(ones_sb[:].bitcast(f32), 1.0)

            def load_xw(j, label, fine=False):
                xw = xw_pool.tile([P, KB, 512], f32r, tag="xw",
                                  name=f"xw_{label}")
                ap = (xT.ap()[:, j * 512:(j + 1) * 512]
                      .rearrange("(kb p) n -> p kb n", p=P).bitcast(f32r))
                if fine:
                    # per-k-block DMAs so matmuls start on partial data
                    for kb in range(KB):
                        nc.sync.dma_start(xw[:, kb, :], ap[:, kb, :])
                else:
                    nc.sync.dma_start(xw[:], ap)
                return xw

            def emit_proj_tiles(xw, w_sb, dst_fn, lbl, copy_fn=None):
                """q/k projection for one 512-window: 4 d'-blocks."""
                for db in range(DB):
                    pq = ps_bg.tile([P, 512], f32, tag="bg",
                                    name=f"pq_{lbl}_{db}")
                    for kb in range(KB):
                        nc.tensor.matmul(
                            pq[:],
                            lhsT=w_sb[:, kb, db * P:(db + 1) * P],
                            rhs=xw[:, kb, :],
                            start=(kb == 0), stop=(kb == KB - 1))
                    if copy_fn is not None:
                        copy_fn(db, pq)
                    else:
                        nc.vector.tensor_copy(out=dst_fn(db), in_=pq[:])

            def emit_v_window(xw, w, wv_sb):
                """v for the 4 m-tiles of window w."""
                for mc in range(4):
                    m = w * 4 + mc
                    pv = ps_bg.tile([P, 512], f32, tag="bg", name=f"pv{m}")
                    for kb in range(KB):
                        nc.tensor.matmul(
                            pv[:],
                            lhsT=xw[:, kb, mc * P:(mc + 1) * P],
                            rhs=wv_sb[:, kb, :],
                            start=(kb == 0), stop=(kb == KB - 1))
                    nc.vector.tensor_copy(
                        out=v_sb[:, m, :, 0:HD],
                        in_=pv[:].rearrange("p (h d) -> p h d", h=NHC))

            qT_tiles = [None] * NCH

            def qt_copy_fn(qt):
                # scores run as K=128 matmuls: per head its q rows live in
                # their own 64 partitions, the other head's 64 rows are zero
                def cp(db, pq):
                    nc.vector.tensor_copy(out=qt[0:HD, 2 * db, :],
                                          in_=pq[0:HD, :])
                    nc.vector.tensor_copy(out=qt[HD:P, 2 * db + 1, :],
                                          in_=pq[HD:P, :])
                return cp

            def emit_qT_chunk(j):
                qt = qT_pool.tile([P, NHC, 512], f32r, tag="qTc", name=f"qT{j}")
                nc.vector.memset(qt[:].bitcast(f32), 0.0)
                xwq = load_xw(j, f"q{j}")
                emit_proj_tiles(xwq, wq_sb, None, f"q{j}",
                                copy_fn=qt_copy_fn(qt))
                qT_tiles[j] = qt

            # ---- prelude: kT + v for all windows, then qT chunk 0 ----
            wkv_scope = tc.tile_pool(name="wkv", bufs=1)
            wkv_pool = wkv_scope.__enter__()
            wk_sb = wkv_pool.tile([P, KB, DC], f32r, tag="wk")
            wv_sb = wkv_pool.tile([P, KB, DC], f32r, tag="wv")
            wk_ap = wkT.ap().rearrange("(kb p) d -> p kb d", p=P).bitcast(f32r)
            wv_ap = wvT.ap().rearrange("(kb p) d -> p kb d", p=P).bitcast(f32r)
            xw0 = xw_pool.tile([P, KB, 512], f32r, tag="xw", name="xw_kv0")
            xw0_ap = (xT.ap()[:, 0:512]
                      .rearrange("(kb p) n -> p kb n", p=P).bitcast(f32r))
            # interleave so the first kT matmul's inputs land first; wv is
            # only needed once kT window 0 is done
            for kb in range(KB):
                nc.sync.dma_start(wk_sb[:, kb, :], wk_ap[:, kb, :])
                nc.sync.dma_start(xw0[:, kb, :], xw0_ap[:, kb, :])
            for kb in range(KB):
                nc.sync.dma_start(wv_sb[:, kb, :], wv_ap[:, kb, :])

            for w in range(NCH):
                xw = xw0 if w == 0 else load_xw(w, f"kv{w}")
                emit_proj_tiles(
                    xw, wk_sb,
                    lambda db, w=w: kT_sb[:, db, w * 512:(w + 1) * 512],
                    f"k{w}")
                emit_v_window(xw, w, wv_sb)
                if w == 0:
                    nc.sync.dma_start(
                        wq_sb[:],
                        wqT.ap().rearrange("(kb p) d -> p kb d",
                                           p=P).bitcast(f32r))
            emit_qT_chunk(0)
            wkv_scope.__exit__(None, None, None)

            expS_scope = tc.tile_pool(name="expS", bufs=5)
            expS_pool = expS_scope.__enter__()
            at_scope = tc.tile_pool(name="at", bufs=2)
            at_pool = at_scope.__enter__()
            small_scope = tc.tile_pool(name="small", bufs=1)
            small_pool = small_scope.__enter__()
            out_scope = tc.tile_pool(name="outsb", bufs=2)
            out_pool = out_scope.__enter__()

            if debug:
                dbg_qT = nc.dram_tensor("dbg_qT", [P, DB, 512], f32, kind="ExternalOutput")
                dbg_kT = nc.dram_tensor("dbg_kT", [P, DB, N], f32, kind="ExternalOutput")
                dbg_v = nc.dram_tensor("dbg_v", [P, MT, NHC, HD + 1], f32, kind="ExternalOutput")
                dbg_at = nc.dram_tensor("dbg_at", [P, DB, 512], f32, kind="ExternalOutput")
                nc.sync.dma_start(dbg_qT.ap(), qT_tiles[0][:].bitcast(f32))

            # ---- attention + projection, per n-chunk ----
            # All score/qT/proj/bcp PSUM traffic shares one 3-deep ring of
            # [128,1024] tiles (6 banks); attn@v accumulators get 2 banks.
            at_tiles = [None] * NCH

            def emit_qT_thunks(j):
                """qT(j) emission as small PE thunks (ring-pool psum)."""
                qt = qT_pool.tile([P, NHC, 512], f32r, tag="qTc", name=f"qT{j}")
                qT_tiles[j] = qt
                xwq = load_xw(j, f"q{j}")
                cpf = qt_copy_fn(qt)
                thunks = [lambda qt=qt: nc.vector.memset(qt[:].bitcast(f32),
                                                         0.0)]
                box = [None]
                for db in range(DB):
                    def mm_t(db, kb0):
                        if kb0 == 0:
                            box[0] = ps_bg.tile([P, 512], f32, tag="bg",
                                                name=f"pqt{db}")
                        for kb in (kb0, kb0 + 1):
                            nc.tensor.matmul(
                                box[0][:],
                                lhsT=wq_sb[:, kb, db * P:(db + 1) * P],
                                rhs=xwq[:, kb, :],
                                start=(kb == 0), stop=(kb == KB - 1))
                    for kb0 in range(0, KB, 2):
                        thunks.append(lambda db=db, kb0=kb0: mm_t(db, kb0))
                    def cp_t(db=db):
                        cpf(db, box[0])
                    thunks.append(cp_t)
                return thunks

            def emit_proj_thunks(j, dual_pool=False):
                """Projection of chunk j as small PE thunks (ring psum).
                dual_pool alternates ps_bg/ps_S tiles (final burst: the S
                ring is idle, 4-way rotation halves the serialization)."""
                at_j = at_tiles[j]
                thunks = []
                box = [None]
                for ns in range(4):
                    for ec in range(2):
                        def mm_t(ns, ec, kb0):
                            if kb0 == 0:
                                if dual_pool and (ns * 2 + ec) % 2 == 1:
                                    box[0] = ps_S.tile(
                                        [P, 1024], f32, tag="S",
                                        name=f"ppt{ns}_{ec}")[:, 0:512]
                                else:
                                    box[0] = ps_bg.tile(
                                        [P, 512], f32, tag="bg",
                                        name=f"ppt{ns}_{ec}")
                            for cb in (kb0, kb0 + 1):
                                nc.tensor.matmul(
                                    box[0][:],
                                    lhsT=at_j[:, cb, ns * P:(ns + 1) * P],
                                    rhs=wp_box[0][:, cb, ec * 512:(ec + 1) * 512],
                                    start=(cb == 0), stop=(cb == DB - 1))
                        for kb0 in range(0, DB, 2):
                            thunks.append(
                                lambda ns=ns, ec=ec, kb0=kb0: mm_t(ns, ec, kb0))
                        def cp_t(ns=ns, ec=ec):
                            osb = out_pool.tile([P, 512], f32, tag="osb",
                                                name=f"osb{ns}_{ec}")
                            nc.vector.tensor_copy(out=osb[:], in_=box[0][:])
                            nc.sync.dma_start(
                                out.ap()[j * 512 + ns * P:j * 512 + (ns + 1) * P,
                                         ec * 512:(ec + 1) * 512],
                                osb[:])
                        thunks.append(cp_t)
                return thunks

            wp_box = [None]

            # ---- one flat step stream across ALL chunks ----
            # S lookahead, attn@v lag and epilogues run continuously across
            # pair AND chunk boundaries; qT(j+1)/proj(j-1) interleave as
            # background thunks (proj delayed past the epilogue flush so
            # at(j-1) is fully written in program order first).
            gsteps = [(j, p, h, i) for j in range(NCH) for p in range(DB)
                      for h in range(2) for i in range(MT // 2)]
            CHUNK = len(gsteps) // NCH
            po_tiles = {}
            eS_q = {}
            pending_p1 = []
            pending_ep = []
            bg_now = []
            bg_later = []
            AV_LAG = 3

            def emit_bg(idx):
                while bg_later and bg_later[0][0] <= idx:
                    bg_now.extend(bg_later.pop(0)[1])
                if bg_now:
                    bg_now.pop(0)()

            def emit_S(j, p, h, i):
                S = ps_S.tile([P, 1024], f32, tag="S", name=f"S{p}_{h}_{i}")
                for half in range(2):
                    m = 2 * i + half
                    nc.tensor.matmul(
                        S[:, half * 512:(half + 1) * 512],
                        lhsT=kT_sb[:, p, m * P:(m + 1) * P],
                        rhs=qT_tiles[j][:, 2 * p + h, :],
                        start=True, stop=True)
                return S

            def emit_epilogue_p1(po_t, h):
                oT = small_pool.tile([HD + 1, 512], f32, tag=f"oT{h}",
                                     name=f"oT{h}")
                nc.vector.tensor_copy(out=oT[:], in_=po_t[0:HD + 1, :])
                rcp = small_pool.tile([HD + 1, 512], f32r, tag="rcp",
                                      name="rcp")
                with nc.allow_low_precision(reason="softmax recip to f32r"):
                    nc.vector.reciprocal(rcp[HD:HD + 1, :], oT[HD:HD + 1, :])
                return oT, rcp

            def emit_epilogue_p2(jj, p, h, oT, rcp):
                at = at_tiles[jj]
                bcp = ps_bg.tile([P, 512], f32, tag="bg", name=f"bcp{h}")
                nc.tensor.matmul(bcp[0:HD, :],
                                 lhsT=ones_sb[HD:HD + 1, :],
                                 rhs=rcp[HD:HD + 1, :],
                                 start=True, stop=True)
                if h == 0:
                    nc.vector.tensor_tensor(
                        out=at[0:HD, p, :], in0=oT[0:HD, :],
                        in1=bcp[0:HD, :], op=MULT)
                else:
                    nc.vector.tensor_tensor(
                        out=oT[0:HD, :], in0=oT[0:HD, :],
                        in1=bcp[0:HD, :], op=MULT)
                    nc.sync.dma_start(at[HD:P, p, :],
                                      oT[0:HD, :].bitcast(f32r))

            def emit_av(idx2):
                jj, pp, ph, pi = gsteps[idx2]
                eSp = eS_q.pop((jj, pp, ph, pi))
                key = (jj, pp)
                if key not in po_tiles:
                    po_tiles[key] = [
                        ps_o.tile([P, 512], f32, tag="o",
                                  name=f"po{jj}_{pp}_{h2}")
                        for h2 in range(2)]
                po_t = po_tiles[key][ph]
                for half in range(2):
                    m = 2 * pi + half
                    nc.tensor.matmul(
                        po_t[0:HD + 1, :],
                        lhsT=v_sb[:, m, 2 * pp + ph, :],
                        rhs=eSp[:, half * 512:(half + 1) * 512],
                        start=(m == 0), stop=(m == MT - 1))
                if pi == MT // 2 - 1:
                    pending_p1.append([3, po_t, jj, pp, ph])

            def drain_queues():
                for ep in pending_p1:
                    ep[0] -= 1
                while pending_p1 and pending_p1[0][0] <= 0:
                    _, po_t, jj, pp, ph = pending_p1.pop(0)
                    oT, rcp = emit_epilogue_p1(po_t, ph)
                    pending_ep.append([4, jj, pp, ph, oT, rcp])
                for ep in pending_ep:
                    ep[0] -= 1
                while pending_ep and pending_ep[0][0] <= 0:
                    _, jj, pp, ph, oT, rcp = pending_ep.pop(0)
                    emit_epilogue_p2(jj, pp, ph, oT, rcp)

            def start_chunk(c, idx):
                if c == 0:
                    wp_scope = tc.tile_pool(name="wp", bufs=1)
                    wp_pool = wp_scope.__enter__()
                    wp_box.append(wp_scope)  # keep scope alive
                    wp_sb = wp_pool.tile([P, DB, D], f32r, tag="wp")
                    nc.sync.dma_start(
                        wp_sb[:],
                        wpT.ap().rearrange("(cb p) e -> p cb e",
                                           p=P).bitcast(f32r))
                    wp_box[0] = wp_sb
                at_tiles[c] = at_pool.tile([P, DB, 512], f32r, tag="at",
                                           name=f"at{c}")
                if c + 1 < NCH:
                    bg_now.extend(emit_qT_thunks(c + 1))
                if c >= 1:
                    bg_later.append((idx + 16, emit_proj_thunks(c - 1)))

            start_chunk(0, 0)
            S_next = emit_S(*gsteps[0])
            for idx, (j, p, h, i) in enumerate(gsteps):
                S_cur = S_next
                eS = expS_pool.tile([P, 1024], f32r, tag="e",
                                    name=f"eS{p}_{h}_{i}")
                nc.scalar.activation(eS[:], S_cur[:], Exp, scale=SCALE)
                eS_q[(j, p, h, i)] = eS
                if idx + 1 < len(gsteps):
                    if (idx + 1) % CHUNK == 0:
                        # entering a new chunk: its qT thunks must be fully
                        # emitted before the lookahead score reads them
                        while bg_now:
                            bg_now.pop(0)()
                        start_chunk((idx + 1) // CHUNK, idx + 1)
                    S_next = emit_S(*gsteps[idx + 1])
                if idx >= AV_LAG:
                    emit_av(idx - AV_LAG)
                drain_queues()
                emit_bg(idx)
                if len(bg_now) > len(gsteps) - idx:
                    emit_bg(idx)

            for idx2 in range(len(gsteps) - AV_LAG, len(gsteps)):
                emit_av(idx2)
            for _, po_t, jj, pp, ph in pending_p1:
                oT, rcp = emit_epilogue_p1(po_t, ph)
                pending_ep.append([0, jj, pp, ph, oT, rcp])
            for _, jj, pp, ph, oT, rcp in pending_ep:
                emit_epilogue_p2(jj, pp, ph, oT, rcp)
            while bg_now or bg_later:
                if not bg_now and bg_later:
                    bg_now.extend(bg_later.pop(0)[1])
                if bg_now:
                    bg_now.pop(0)()

            # final chunk's projection
            for t in emit_proj_thunks(NCH - 1, dual_pool=True):
                t()

            if len(wp_box) > 1:
                wp_box[1].__exit__(None, None, None)
            out_scope.__exit__(None, None, None)
            small_scope.__exit__(None, None, None)
            at_scope.__exit__(None, None, None)
            expS_scope.__exit__(None, None, None)

    nc.compile()
    return nc


_CACHE: dict = {}


def _get_program():
    if "nc" not in _CACHE:
        _CACHE["nc"] = build_program()
    return _CACHE["nc"]


def make_in_maps(x, w_qkv, w_proj):
    """Host-side sharding: per-core input dict."""
    x = np.ascontiguousarray(np.asarray(x, dtype=np.float32))
    w_qkv = np.asarray(w_qkv, dtype=np.float32)
    w_proj = np.asarray(w_proj, dtype=np.float32)
    in_maps = []
    for core in range(8):
        b, g = divmod(core, 2)
        gsl = slice(g * DC, (g + 1) * DC)
        in_maps.append({
            "xT": np.ascontiguousarray(x[b].T),                       # [D, N]
            "wqT": np.ascontiguousarray(w_qkv[0 * D:1 * D][gsl].T),   # [D, DC]
            "wkT": np.ascontiguousarray(w_qkv[1 * D:2 * D][gsl].T),
            "wvT": np.ascontiguousarray(w_qkv[2 * D:3 * D][gsl].T),
            "wpT": np.ascontiguousarray(w_proj[:, gsl].T),            # [DC, D]
        })
    return in_maps


def run(x, w_qkv, w_proj, b_proj, **spmd_kwargs):
    nc = _get_program()
    in_maps = make_in_maps(x, w_qkv, w_proj)
    res = run_bass_kernel_spmd(nc, in_maps, list(range(8)), **spmd_kwargs)
    b_proj = np.asarray(b_proj, dtype=np.float32)
    outp = np.empty((B, N, D), dtype=np.float32)
    for b in range(B):
        outp[b] = (res.results[2 * b]["out"] + res.results[2 * b + 1]["out"]
                   + b_proj[None, :])
    return outp, res


def kernel(x, w_qkv, w_proj, b_proj):
    outp, _ = run(x, w_qkv, w_proj, b_proj)
    return outp
